# revision 2
# baseline (speedup 1.0000x reference)
"""DIEN forward-loss kernel for Trainium2, SPMD over 8 NeuronCores. V2.

Data-parallel over batch (32 rows/core), embedding replicated. Differences
from V1: bf16 matmuls/tensors in the recurrence (fp32 matmuls lower to two
passes and disable fast weight load), x-side preactivations + gate biases
accumulated directly in PSUM chunk banks by the tensor engine (no per-step
DVE adds for the gates), z-gate negated host-side so GRU and AUGRU share the
update form h += g*(T - h), n-gate path uses scalar_tensor_tensor fused ops,
GRU+AUGRU tanh fused into one ACT op, embedding gather/renorm pipelined into
the recurrence slot loop, and a transposed packed AllGather payload so the
final MLP needs no PE transposes or per-core scalar DMAs.

PSUM banks: RZ(x2) GRU [zbar|r] chunk preacts, NX(x2) GRU [xn|nh],
UR(x2) AUGRU [u|r], XH(x1) AUGRU [xh|ah], shared(x1) aux-gram/transposes.
"""
import numpy as np
import concourse.bass as bass
import concourse.bacc as bacc
import concourse.mybir as mybir
import concourse.tile as tile
from concourse.bass_utils import run_bass_kernel_spmd
from concourse.masks import make_identity

F32 = mybir.dt.float32
BF16 = mybir.dt.bfloat16
I32 = mybir.dt.int32
AF = mybir.ActivationFunctionType
OP = mybir.AluOpType

B, L, D, NV = 256, 200, 128, 500000
NCORES = 8
BL = B // NCORES          # 32 batch rows per core
NT = L * BL               # 6400 (t,b) pairs per core
NTIL = NT // 128          # 50 gather tiles
CH = 8                    # timesteps per chunk
CW = CH * BL              # 256 cols per chunk gate region
NCHUNK = L // CH          # 25
LAG = 12                  # AUGRU lags GRU by 12 steps (1.5 chunks)
NSLOT = L + LAG           # 212
TLEAD = 6                 # tile k processed at slot 4*(k-TLEAD)
EPS_BN = 1e-5
DICE_A = 0.1
ALPHA = 0.2
MAGIC = 0x5F3759DF


def _rsqrt(nc, pool, v, out, shape, iters=3):
    """out = 1/sqrt(v) on DVE (quake seed + Newton). v >= 0."""
    p, n = shape
    iv = out.bitcast(I32)
    nc.vector.tensor_scalar(out=iv, in0=v.bitcast(I32), scalar1=1,
                            scalar2=None, op0=OP.arith_shift_right)
    nc.vector.tensor_scalar(out=iv, in0=iv, scalar1=-1, scalar2=None,
                            op0=OP.bitwise_xor)
    nc.vector.tensor_scalar(out=iv, in0=iv, scalar1=MAGIC + 1, scalar2=None,
                            op0=OP.add)
    t = pool.tile([p, n], F32, tag="rsqrt_t")
    for _ in range(iters):
        nc.vector.tensor_tensor(out=t[:], in0=v, in1=out, op=OP.mult)
        nc.vector.tensor_tensor(out=t[:], in0=t[:], in1=out, op=OP.mult)
        nc.vector.tensor_scalar(out=t[:], in0=t[:], scalar1=-0.5, scalar2=1.5,
                                op0=OP.mult, op1=OP.add)
        nc.vector.tensor_tensor(out=out, in0=out, in1=t[:], op=OP.mult)


def build_bass(upto="full"):
    nc = bacc.Bacc("TRN2", target_bir_lowering=False, num_devices=NCORES)

    # ---------------- kernel parameters ----------------
    emb = nc.declare_dram_parameter("emb", [NV, D], F32, isOutput=False)
    idx_h = nc.declare_dram_parameter("idx_h", [128, NTIL], I32, isOutput=False)
    y_h = nc.declare_dram_parameter("y_h", [128, NTIL], F32, isOutput=False)
    idx_t = nc.declare_dram_parameter("idx_t", [BL, 1], I32, isOutput=False)
    # GRU weights (bf16): x-side / h-side for [zbar | r] and n
    wgx = nc.declare_dram_parameter("wgx", [D, 2 * D], BF16, isOutput=False)
    wgh = nc.declare_dram_parameter("wgh", [D, 2 * D], BF16, isOutput=False)
    wnx = nc.declare_dram_parameter("wnx", [D, D], BF16, isOutput=False)
    wnh = nc.declare_dram_parameter("wnh", [D, D], BF16, isOutput=False)
    bg_row = nc.declare_dram_parameter("bg_row", [1, 2 * D], BF16, isOutput=False)
    bihn_r = nc.declare_dram_parameter("bihn_r", [1, D], BF16, isOutput=False)
    bhhn_c = nc.declare_dram_parameter("bhhn_c", [D, 1], F32, isOutput=False)
    # AUGRU weights (bf16): [u | r] x/h side and h-candidate
    aux_w = nc.declare_dram_parameter("aux_w", [D, 2 * D], BF16, isOutput=False)
    auh_w = nc.declare_dram_parameter("auh_w", [D, 2 * D], BF16, isOutput=False)
    ahx_w = nc.declare_dram_parameter("ahx_w", [D, D], BF16, isOutput=False)
    ahh_w = nc.declare_dram_parameter("ahh_w", [D, D], BF16, isOutput=False)
    ba_row = nc.declare_dram_parameter("ba_row", [1, 2 * D], BF16, isOutput=False)
    bh_r = nc.declare_dram_parameter("bh_r", [1, D], BF16, isOutput=False)
    # final MLP (fp32)
    W1a = nc.declare_dram_parameter("W1a", [D, D], F32, isOutput=False)
    W1b = nc.declare_dram_parameter("W1b", [D, D], F32, isOutput=False)
    b1 = nc.declare_dram_parameter("b1", [1, D], F32, isOutput=False)
    W2 = nc.declare_dram_parameter("W2", [D, D // 2], F32, isOutput=False)
    b2 = nc.declare_dram_parameter("b2", [1, D // 2], F32, isOutput=False)
    Wf = nc.declare_dram_parameter("Wf", [D // 2, 1], F32, isOutput=False)
    bf = nc.declare_dram_parameter("bf", [1, 1], F32, isOutput=False)
    h0T = nc.declare_dram_parameter("h0T", [D, BL], BF16, isOutput=False)
    y_t = nc.declare_dram_parameter("y_t", [1, B], F32, isOutput=False)
    out_p = nc.declare_dram_parameter("out", [1, 1], F32, isOutput=True)

    PW = 2 * BL + 2            # 66: [hT(32) | itemT(32) | aux | pad]
    ploc = nc.dram_tensor("ploc", [D, PW], F32)
    gall = nc.dram_tensor("gall", [NCORES * D, PW], F32)

    with tile.TileContext(nc) as tc:
        with (
            tc.tile_pool(name="persist", bufs=1) as pp,
            tc.tile_pool(name="work", bufs=3) as wk,
            tc.tile_pool(name="ebuf", bufs=3) as eb,
            tc.tile_pool(name="xchunk", bufs=2) as xc,
            tc.tile_pool(name="ps_ck", bufs=2, space="PSUM") as pck,
            tc.tile_pool(name="ps_pg", bufs=2, space="PSUM") as ppg,
            tc.tile_pool(name="ps_png", bufs=1, space="PSUM") as ppng,
            tc.tile_pool(name="ps_pa", bufs=2, space="PSUM") as ppa,
            tc.tile_pool(name="ps_pna", bufs=1, space="PSUM") as ppna,
        ):
            # ---------------- constants / weights ----------------
            ident = pp.tile([128, 128], F32, tag="ident")
            make_identity(nc, ident[:])
            identb = pp.tile([128, 128], BF16, tag="identb")
            nc.vector.tensor_copy(identb[:], ident[:])
            ones_row = pp.tile([1, CW], BF16, tag="ones_row")
            nc.gpsimd.memset(ones_row[:], 1.0)
            ones_b = pp.tile([1, B], F32, tag="ones_b")
            nc.gpsimd.memset(ones_b[:], 1.0)
            ones_col = pp.tile([128, 1], F32, tag="ones_col")
            nc.gpsimd.memset(ones_col[:], 1.0)

            def load(ap, shape, tag, dt=F32):
                t = pp.tile(shape, dt, tag=tag)
                nc.sync.dma_start(out=t[:], in_=ap[:])
                return t

            wgx_s = load(wgx, [D, 2 * D], "wgx", BF16)
            wgh_s = load(wgh, [D, 2 * D], "wgh", BF16)
            wnx_s = load(wnx, [D, D], "wnx", BF16)
            wnh_s = load(wnh, [D, D], "wnh", BF16)
            bg_s = load(bg_row, [1, 2 * D], "bg", BF16)
            aux_s = load(aux_w, [D, 2 * D], "aux", BF16)
            auh_s = load(auh_w, [D, 2 * D], "auh", BF16)
            ahx_s = load(ahx_w, [D, D], "ahx", BF16)
            ahh_s = load(ahh_w, [D, D], "ahh", BF16)
            ba_s = load(ba_row, [1, 2 * D], "ba", BF16)
            bihn_s = load(bihn_r, [1, D], "bihn", BF16)
            bhhn_c_s = load(bhhn_c, [D, 1], "bhhn_c")
            bh_s = load(bh_r, [1, D], "bh", BF16)
            W1a_s = load(W1a, [D, D], "W1a")
            W1b_s = load(W1b, [D, D], "W1b")
            b1_s = load(b1, [1, D], "b1")
            W2_s = load(W2, [D, D // 2], "W2")
            b2_s = load(b2, [1, D // 2], "b2")
            Wf_s = load(Wf, [D // 2, 1], "Wf")
            bf_s = load(bf, [1, 1], "bf")
            y_t_s = load(y_t, [1, B], "y_t")
            y_h_s = load(y_h, [128, NTIL], "y_h")
            h0s = pp.tile([D, BL], BF16, tag="h0s")
            nc.sync.dma_start(out=h0s[:], in_=h0T[:])

            idx_s = pp.tile([128, NTIL], I32, tag="idx_s")
            nc.sync.dma_start(out=idx_s[:], in_=idx_h[:])
            idx_t_s = pp.tile([BL, 1], I32, tag="idx_t_s")
            nc.sync.dma_start(out=idx_t_s[:], in_=idx_t[:])

            # persistent big buffers
            ET = pp.tile([128, NT], BF16, tag="ET")
            X = pp.tile([128, (NSLOT + 1) * 2 * BL], BF16, tag="X")
            outsT = pp.tile([128, NT], BF16, tag="outsT")
            erows = pp.tile([128, NT], F32, tag="erows")
            ss_all = pp.tile([128, NTIL], F32, tag="ss_all")
            sc_all = pp.tile([128, NTIL], F32, tag="sc_all")
            s_all = pp.tile([128, NTIL], F32, tag="s_all")

            # ---------- all embedding gathers issued upfront ----------
            # target item first: its consumers run at the very end, but a
            # late gather would trap the whole pipeline behind it.
            itemr = pp.tile([BL, D], F32, tag="itemr")
            nc.gpsimd.indirect_dma_start(
                out=itemr[:], out_offset=None, in_=emb[:],
                in_offset=bass.IndirectOffsetOnAxis(ap=idx_t_s[:, :1], axis=0),
            )
            for k in range(NTIL):
                nc.gpsimd.indirect_dma_start(
                    out=erows[:, 128 * k:128 * (k + 1)], out_offset=None,
                    in_=emb[:],
                    in_offset=bass.IndirectOffsetOnAxis(ap=idx_s[:, k:k + 1],
                                                        axis=0),
                )

            def do_item():
                # target item renorm (fp32, rows) + transpose to [D, BL]
                sqt = wk.tile([BL, D], F32, tag="sqt")
                sst = wk.tile([BL, 1], F32, tag="sst")
                nc.scalar.activation(sqt[:], itemr[:], AF.Square,
                                     accum_out=sst[:])
                sct = wk.tile([BL, 1], F32, tag="sct")
                _rsqrt(nc, wk, sst[:], sct[:], [BL, 1])
                nc.vector.tensor_scalar_min(out=sct[:], in0=sct[:], scalar1=1.0)
                nc.vector.tensor_scalar(out=itemr[:], in0=itemr[:],
                                        scalar1=sct[:], scalar2=None,
                                        op0=OP.mult)
                itT_ps = pck.tile([D, BL], F32, tag="ck")
                nc.tensor.transpose(out=itT_ps[:], in_=itemr[:],
                                    identity=ident[0:BL, 0:BL])
                itemT = pp.tile([D, BL], F32, tag="itemT")
                nc.vector.tensor_copy(itemT[:], itT_ps[:])
                return itemT

            # ---------- per-tile embedding pipeline (phase A) ----------
            def do_tile(k):
                # norm accumulation for tile k; rsqrt batched per 4 tiles
                er = erows[:, 128 * k:128 * (k + 1)]
                sq = wk.tile([128, 128], F32, tag="sq_scr")
                nc.vector.scalar_tensor_tensor(
                    out=sq[:], in0=er, scalar=0.0, in1=er,
                    op0=OP.add, op1=OP.mult, accum_out=ss_all[:, k:k + 1])
                if k % 4 == 3 or k == NTIL - 1:
                    k0 = (k // 4) * 4
                    w = k - k0 + 1
                    _rsqrt(nc, wk, ss_all[:, k0:k + 1], sc_all[:, k0:k + 1],
                           [128, w], iters=1)
                    nc.vector.tensor_scalar_min(out=sc_all[:, k0:k + 1],
                                                in0=sc_all[:, k0:k + 1],
                                                scalar1=1.0)

            def do_tile2(j):
                # renorm-scale + transpose tile j into ET (rsqrt must be done)
                erj = erows[:, 128 * j:128 * (j + 1)]
                ersc = eb.tile([128, 128], F32, tag="ersc")
                nc.scalar.activation(ersc[:], erj, AF.Copy,
                                     scale=sc_all[:, j:j + 1])
                tp = pck.tile([128, 128], F32, tag="ck")
                nc.tensor.transpose(out=tp[:], in_=ersc[:], identity=ident[:])
                nc.vector.tensor_copy(ET[:, 128 * j:128 * (j + 1)], tp[:])

            for k in range(TLEAD):
                do_tile(k)
            for j in range(4):
                do_tile2(j)

            if upto == "A":
                for k in range(TLEAD, NTIL):
                    do_tile(k)
                for j in range(4, NTIL):
                    do_tile2(j)
                dbg = wk.tile([1, 1], F32, tag="res")
                nc.vector.reduce_sum(out=dbg[:], in_=ET[0:1, 0:128],
                                     axis=mybir.AxisListType.X)
                nc.sync.dma_start(out=out_p[:], in_=dbg[:])

            # ---------- recurrence slot loop ----------
            def emit_g_chunk(c):
                # x-side preacts for GRU chunk c -> PSUM (closed) -> SBUF bf16
                ecols = ET[:, c * CW:(c + 1) * CW]
                ck = pck.tile([128, 2 * CW], F32, tag="ck")
                nc.tensor.matmul(ck[:, 0:CW], wgx_s[:, 0:D], ecols,
                                 start=True, stop=False)
                nc.tensor.matmul(ck[:, CW:2 * CW], wgx_s[:, D:2 * D],
                                 ecols, start=False, stop=False)
                nc.tensor.matmul(ck[:, 0:CW], bg_s[0:1, 0:D],
                                 ones_row[0:1, :], start=False, stop=False)
                nc.tensor.matmul(ck[:, CW:2 * CW], bg_s[0:1, D:2 * D],
                                 ones_row[0:1, :], start=False, stop=True)
                gxrz_t = xc.tile([128, 2 * CW], BF16, tag="gxrz")
                nc.vector.tensor_copy(gxrz_t[:], ck[:])
                ck2 = pck.tile([128, 2 * CW], F32, tag="ck")
                nc.tensor.matmul(ck2[:, 0:CW], wnx_s[:], ecols,
                                 start=True, stop=False)
                nc.tensor.matmul(ck2[:, 0:CW], bihn_s[0:1, :],
                                 ones_row[0:1, :], start=False, stop=True)
                gxn_t = xc.tile([128, CW], BF16, tag="gxn")
                nc.scalar.activation(gxn_t[:], ck2[:, 0:CW], AF.Copy)
                return gxrz_t, gxn_t

            def emit_a_chunk(ca):
                ocols = outsT[:, ca * CW:(ca + 1) * CW]
                ck = pck.tile([128, 2 * CW], F32, tag="ck")
                nc.tensor.matmul(ck[:, 0:CW], aux_s[:, 0:D], ocols,
                                 start=True, stop=False)
                nc.tensor.matmul(ck[:, CW:2 * CW], aux_s[:, D:2 * D],
                                 ocols, start=False, stop=False)
                nc.tensor.matmul(ck[:, 0:CW], ba_s[0:1, 0:D],
                                 ones_row[0:1, :], start=False, stop=False)
                nc.tensor.matmul(ck[:, CW:2 * CW], ba_s[0:1, D:2 * D],
                                 ones_row[0:1, :], start=False, stop=True)
                axur_t = xc.tile([128, 2 * CW], BF16, tag="axur")
                nc.vector.tensor_copy(axur_t[:], ck[:])
                ck2 = pck.tile([128, 2 * CW], F32, tag="ck")
                nc.tensor.matmul(ck2[:, 0:CW], ahx_s[:], ocols,
                                 start=True, stop=False)
                nc.tensor.matmul(ck2[:, 0:CW], bh_s[0:1, :],
                                 ones_row[0:1, :], start=False, stop=True)
                axh_t = xc.tile([128, CW], BF16, tag="axh")
                nc.scalar.activation(axh_t[:], ck2[:, 0:CW], AF.Copy)
                return axur_t, axh_t

            gxrz = gxn = axur = axh = None
            gxrz_nx = gxn_nx = axur_nx = axh_nx = None
            if upto != "A":
                # PE warm-up: a dense burst of matmuls latches the HAM
                # clock-gate to K=8/8 (2.4 GHz) while the first gathers are
                # still landing; in-loop PE gaps are far shorter than the
                # ~3.4us idle window needed to re-throttle.
                wmm = pck.tile([128, 2 * CW], F32, tag="ck")
                for _ in range(60):
                    nc.tensor.matmul(wmm[:, 0:2 * D], wgx_s[:, 0:D],
                                     wgx_s[:], start=True, stop=True)
                gxrz_nx, gxn_nx = emit_g_chunk(0)
                # X state ring: X[i] = [hG(i-1) | hA(i-1-LAG)] at cols i*2BL.
                X_v = X[:].rearrange("p (s w) -> p s w", w=2 * BL)
                nc.gpsimd.memset(X[:, 0:2 * BL], 0.0)

                def xg_view(t0, n):
                    # [128, n, BL] strided view of hG(t0..t0+n-1)
                    return X_v[:, t0 + 1:t0 + 1 + n, 0:BL]

                for s in range(NSLOT):
                    tg = s
                    ta = s - LAG
                    # phase-A tile interleave
                    if s % 4 == 0:
                        m = s // 4
                        if m + TLEAD < NTIL:
                            do_tile(m + TLEAD)
                        if 4 <= m + 2 < NTIL:
                            do_tile2(m + 2)
                    if s == CH:
                        # re-latch the PE clock gate now that the pipeline is
                        # dense: one ~4us contiguous matmul burst flips HAM to
                        # K=8/8; the steady-state stream never idles >3.4us,
                        # so the warm state then persists.
                        wm2 = pck.tile([128, 2 * CW], F32, tag="ck")
                        for _ in range(12):
                            nc.tensor.matmul(wm2[:], wgx_s[:, 0:D],
                                             ET[:, 0:2 * CW],
                                             start=True, stop=True)
                    # rotate GRU chunk buffers; prefetch next chunk mid-chunk
                    if tg < L and tg % CH == 0:
                        gxrz, gxn = gxrz_nx, gxn_nx
                    if tg % CH == 4 and tg // CH + 1 < NCHUNK:
                        gxrz_nx, gxn_nx = emit_g_chunk(tg // CH + 1)
                    # AUGRU chunks prefetched 3 slots before first use
                    if s >= 9 and (s - 9) % CH == 0 and (s - 9) // CH < NCHUNK:
                        axur_nx, axh_nx = emit_a_chunk((s - 9) // CH)
                    if 0 <= ta < L and ta % CH == 0:
                        axur, axh = axur_nx, axh_nx
                    if s == LAG:
                        # hA(-1) = h0 lands in the A-half of X[LAG]
                        nc.vector.tensor_copy(X_v[:, LAG, BL:2 * BL], h0s[:])
                    h_prev = X_v[:, s, 0:BL]
                    hA_prev = X_v[:, s, BL:2 * BL]
                    # ---- per-step work ----
                    t2 = wk.tile([128, 2 * BL], BF16, tag="t2")
                    gT = wk.tile([128, 2 * BL], BF16, tag="gT")
                    gp = wk.tile([128, 4 * BL], BF16, tag="gp")
                    if tg < L:
                        o = tg % CH
                        sl = slice(o * BL, (o + 1) * BL)
                        # gate preacts: x via identity-inject + h matmuls
                        pgt = ppg.tile([128, 2 * BL], F32, tag="pg")
                        gx_v = gxrz[:].rearrange("p (g w) -> p g w", g=2)
                        pg_v = pgt[:].rearrange("p (g w) -> p g w", g=2)
                        nc.tensor.matmul(pg_v[:, :, :], identb[:], gx_v[:, :, sl],
                                         start=True, stop=False)
                        nc.tensor.matmul(pgt[:, 0:BL], wgh_s[:, 0:D], h_prev,
                                         start=False, stop=False)
                        nc.tensor.matmul(pgt[:, BL:2 * BL], wgh_s[:, D:2 * D],
                                         h_prev, start=False, stop=True)
                        nc.scalar.activation(gp[:, 0:2 * BL], pgt[:], AF.Sigmoid)
                        pngt = ppng.tile([128, BL], F32, tag="png")
                        nc.tensor.matmul(pngt[:], wnh_s[:], h_prev,
                                         start=True, stop=True)
                        # t = (nh + bhhn) * r ; t2 = t + (xn + bihn)
                        tg1 = wk.tile([128, BL], BF16, tag="tg1")
                        nc.vector.scalar_tensor_tensor(
                            out=tg1[:], in0=pngt[:], scalar=bhhn_c_s[:, 0:1],
                            in1=gp[:, BL:2 * BL], op0=OP.add, op1=OP.mult)
                        nc.vector.tensor_tensor(out=t2[:, 0:BL], in0=tg1[:],
                                                in1=gxn[:, sl], op=OP.add)
                    # ---- AUGRU per-step ----
                    if 0 <= ta < L:
                        o2 = ta % CH
                        sl = slice(o2 * BL, (o2 + 1) * BL)
                        pat = ppa.tile([128, 2 * BL], F32, tag="pa")
                        ax_v = axur[:].rearrange("p (g w) -> p g w", g=2)
                        pa_v = pat[:].rearrange("p (g w) -> p g w", g=2)
                        nc.tensor.matmul(pa_v[:, :, :], identb[:], ax_v[:, :, sl],
                                         start=True, stop=False)
                        nc.tensor.matmul(pat[:, 0:BL], auh_s[:, 0:D], hA_prev,
                                         start=False, stop=False)
                        nc.tensor.matmul(pat[:, BL:2 * BL], auh_s[:, D:2 * D],
                                         hA_prev, start=False, stop=True)
                        nc.scalar.activation(gp[:, 2 * BL:4 * BL], pat[:],
                                             AF.Sigmoid)
                        pnat = ppna.tile([128, BL], F32, tag="pna")
                        nc.tensor.matmul(pnat[:], ahh_s[:], hA_prev,
                                         start=True, stop=True)
                        ta1 = wk.tile([128, BL], BF16, tag="ta1")
                        nc.vector.tensor_tensor(out=ta1[:], in0=pnat[:],
                                                in1=gp[:, 3 * BL:4 * BL],
                                                op=OP.mult)
                        nc.vector.tensor_tensor(out=t2[:, BL:2 * BL], in0=ta1[:],
                                                in1=axh[:, sl], op=OP.add)
                    # ---- fused tanh + updates: h' = h + g*(T - h) ----
                    # gp layout: [gG | rG | uA | rA]; update gates = [gG | uA]
                    gup = gp[:].rearrange("p (a b w) -> p a b w", a=2, b=2)
                    if tg < L and 0 <= ta:
                        nc.scalar.activation(gT[:], t2[:], AF.Tanh)
                        d1 = wk.tile([128, 2 * BL], BF16, tag="d1")
                        nc.vector.tensor_tensor(out=d1[:], in0=gT[:],
                                                in1=X_v[:, s, :], op=OP.subtract)
                        d2 = wk.tile([128, 2 * BL], BF16, tag="d2")
                        d2v = d2[:].rearrange("p (b w) -> p b w", b=2)
                        nc.vector.tensor_tensor(out=d2v, in0=gup[:, :, 0, :],
                                                in1=d1[:].rearrange(
                                                    "p (b w) -> p b w", b=2),
                                                op=OP.mult)
                        nc.vector.tensor_tensor(out=X_v[:, s + 1, :],
                                                in0=X_v[:, s, :], in1=d2[:],
                                                op=OP.add)
                    elif tg < L:
                        nc.scalar.activation(gT[:, 0:BL], t2[:, 0:BL], AF.Tanh)
                        d1 = wk.tile([128, 2 * BL], BF16, tag="d1")
                        nc.vector.tensor_tensor(out=d1[:, 0:BL], in0=gT[:, 0:BL],
                                                in1=h_prev, op=OP.subtract)
                        d2 = wk.tile([128, 2 * BL], BF16, tag="d2")
                        nc.vector.tensor_tensor(out=d2[:, 0:BL],
                                                in0=gp[:, 0:BL],
                                                in1=d1[:, 0:BL], op=OP.mult)
                        nc.vector.tensor_tensor(out=X_v[:, s + 1, 0:BL],
                                                in0=h_prev, in1=d2[:, 0:BL],
                                                op=OP.add)
                    else:
                        nc.scalar.activation(gT[:, BL:2 * BL], t2[:, BL:2 * BL],
                                             AF.Tanh)
                        d1 = wk.tile([128, 2 * BL], BF16, tag="d1")
                        nc.vector.tensor_tensor(out=d1[:, BL:2 * BL],
                                                in0=gT[:, BL:2 * BL],
                                                in1=hA_prev, op=OP.subtract)
                        d2 = wk.tile([128, 2 * BL], BF16, tag="d2")
                        nc.vector.tensor_tensor(out=d2[:, BL:2 * BL],
                                                in0=gp[:, 2 * BL:3 * BL],
                                                in1=d1[:, BL:2 * BL], op=OP.mult)
                        nc.vector.tensor_tensor(out=X_v[:, s + 1, BL:2 * BL],
                                                in0=hA_prev,
                                                in1=d2[:, BL:2 * BL], op=OP.add)
                    if tg < L:
                        # contiguous mirror of hG for the AUGRU x-chunk and
                        # aux-gram matmuls (strided matmul operands are slow)
                        nc.vector.tensor_copy(outsT[:, tg * BL:(tg + 1) * BL],
                                              X_v[:, s + 1, 0:BL])
                    # ---- aux gram: diag(E_blk^T @ outs_blk) ----
                    if tg - 2 >= 0 and (tg - 2) % 4 == 3 and tg - 2 < L:
                        blk = (tg - 2) // 4
                        gps = pck.tile([128, 128], F32, tag="ck")
                        nc.tensor.matmul(gps[:],
                                         ET[:, 128 * blk:128 * (blk + 1)],
                                         outsT[:, 128 * blk:128 * (blk + 1)],
                                         start=True, stop=True)
                        gsc = wk.tile([128, 128], F32, tag="gram_scr")
                        nc.vector.scalar_tensor_tensor(
                            out=gsc[:], in0=gps[:], scalar=1.0, in1=ident[:],
                            op0=OP.mult, op1=OP.mult,
                            accum_out=s_all[:, blk:blk + 1])

            if upto == "G":
                dbg = wk.tile([1, 1], F32, tag="res")
                dbf = wk.tile([1, 128], F32, tag="resb")
                nc.vector.tensor_copy(dbf[:], X[0:1, (NSLOT + 1) * 2 * BL - 128:])
                nc.vector.reduce_sum(out=dbg[:], in_=dbf[:],
                                     axis=mybir.AxisListType.X)
                nc.sync.dma_start(out=out_p[:], in_=dbg[:])
            if upto == "GA":
                dbg = wk.tile([1, 1], F32, tag="res")
                dbf = wk.tile([1, BL], F32, tag="resb")
                nc.vector.tensor_copy(dbf[:], X_v[0:1, NSLOT, BL:2 * BL])
                nc.vector.reduce_sum(out=dbg[:], in_=dbf[:],
                                     axis=mybir.AxisListType.X)
                nc.sync.dma_start(out=out_p[:], in_=dbg[:])

            if upto in ("X", "full"):
                # ---------- aux BCE partial (exp/ln table set) ----------
                ebuf = pp.tile([128, NTIL], F32, tag="ebuf")
                nc.scalar.activation(ebuf[:], s_all[:], AF.Exp)
                nc.vector.tensor_scalar_add(out=ebuf[:], in0=ebuf[:], scalar1=1.0)
                sp = pp.tile([128, NTIL], F32, tag="sp")
                nc.scalar.activation(sp[:], ebuf[:], AF.Ln)
                spm = pp.tile([128, NTIL], F32, tag="spm")
                nc.vector.tensor_tensor(out=spm[:], in0=sp[:], in1=s_all[:],
                                        op=OP.subtract)
                nc.vector.tensor_scalar_min(out=spm[:], in0=spm[:], scalar1=100.0)
                nc.vector.tensor_scalar_min(out=sp[:], in0=sp[:], scalar1=100.0)
                nc.vector.tensor_tensor(out=spm[:], in0=spm[:], in1=sp[:],
                                        op=OP.subtract)
                nc.vector.tensor_tensor(out=spm[:], in0=y_h_s[:], in1=spm[:],
                                        op=OP.mult)
                nc.vector.tensor_tensor(out=sp[:], in0=sp[:], in1=spm[:],
                                        op=OP.add)
                rsum = wk.tile([128, 1], F32, tag="rsum")
                nc.vector.reduce_sum(out=rsum[:], in_=sp[:],
                                     axis=mybir.AxisListType.X)
                aux_ps = pck.tile([1, 1], F32, tag="ck")
                nc.tensor.matmul(aux_ps[:], rsum[:], ones_col[:, 0:1],
                                 start=True, stop=True)

                # ---------- pack (transposed) + AllGather ----------
                itemT = do_item()
                stage = pp.tile([D, PW], F32, tag="stage")
                nc.gpsimd.memset(stage[:], 0.0)
                nc.vector.tensor_copy(stage[:, 0:BL], X_v[:, NSLOT, BL:2 * BL])
                nc.vector.tensor_copy(stage[:, BL:2 * BL], itemT[:])
                nc.vector.tensor_copy(stage[0:1, 2 * BL:2 * BL + 1], aux_ps[:])
                nc.sync.dma_start(out=ploc[:], in_=stage[:])
                nc.gpsimd.collective_compute(
                    "AllGather", OP.bypass,
                    replica_groups=[list(range(NCORES))],
                    ins=[ploc[:]], outs=[gall[:]],
                )

            if upto == "X":
                dbg = wk.tile([1, 1], F32, tag="res")
                nc.vector.tensor_copy(dbg[:], aux_ps[:])
                nc.sync.dma_start(out=out_p[:], in_=dbg[:])

            if upto == "full":
                # ---------- replicated final MLP ----------
                gat = pp.tile([D, NCORES * PW], F32, tag="gat")
                for c in range(NCORES):
                    nc.sync.dma_start(out=gat[:, c * PW:(c + 1) * PW],
                                      in_=gall[c * D:(c + 1) * D, :])
                gat_v = gat[:].rearrange("p (c w) -> p c w", c=NCORES)
                hT_v = gat_v[:, :, 0:BL]            # [128, 8, 32]
                iT_v = gat_v[:, :, BL:2 * BL]
                aux_v = gat_v[0:1, :, 2 * BL:2 * BL + 1]   # [1, 8, 1]
                aux8 = wk.tile([1, NCORES], F32, tag="aux8")
                aux8_v = aux8[:].rearrange("p (c w) -> p c w", w=1)
                nc.vector.tensor_copy(aux8_v, aux_v)
                aux_tot = wk.tile([1, 1], F32, tag="aux_tot")
                nc.vector.reduce_sum(out=aux_tot[:], in_=aux8[:],
                                     axis=mybir.AxisListType.X)

                def dice(z_ps, pdim):
                    m = wk.tile([pdim, 1], F32, tag="dice_m")
                    nc.vector.reduce_sum(out=m[:], in_=z_ps[:],
                                         axis=mybir.AxisListType.X)
                    nc.vector.tensor_scalar_mul(out=m[:], in0=m[:],
                                                scalar1=1.0 / B)
                    xc = wk.tile([pdim, B], F32, tag="dice_xc")
                    nc.vector.tensor_scalar(out=xc[:], in0=z_ps[:], scalar1=m[:],
                                            scalar2=None, op0=OP.subtract)
                    sq2 = wk.tile([pdim, B], F32, tag="dice_sq")
                    vs = wk.tile([pdim, 1], F32, tag="dice_vs")
                    nc.scalar.activation(sq2[:], xc[:], AF.Square,
                                         accum_out=vs[:])
                    nc.vector.tensor_scalar(out=vs[:], in0=vs[:], scalar1=1.0 / B,
                                            scalar2=EPS_BN, op0=OP.mult,
                                            op1=OP.add)
                    inv = wk.tile([pdim, 1], F32, tag="dice_inv")
                    _rsqrt(nc, wk, vs[:], inv[:], [pdim, 1])
                    pr = wk.tile([pdim, B], F32, tag="dice_p")
                    nc.scalar.activation(pr[:], xc[:], AF.Sigmoid,
                                         scale=inv[:, 0:1])
                    nc.vector.tensor_scalar(out=pr[:], in0=pr[:],
                                            scalar1=1 - DICE_A, scalar2=DICE_A,
                                            op0=OP.mult, op1=OP.add)
                    zd = wk.tile([pdim, B], F32, tag="dice_zd")
                    nc.vector.tensor_tensor(out=zd[:], in0=z_ps[:], in1=pr[:],
                                            op=OP.mult)
                    return zd

                z1_ps = pck.tile([128, B], F32, tag="ck")
                nc.tensor.matmul(z1_ps[:], W1a_s[:], hT_v,
                                 start=True, stop=False)
                nc.tensor.matmul(z1_ps[:], W1b_s[:], iT_v,
                                 start=False, stop=False)
                nc.tensor.matmul(z1_ps[:], b1_s[0:1, :], ones_b[0:1, :],
                                 start=False, stop=True)
                z1d = dice(z1_ps, 128)

                z2_ps = pck.tile([D // 2, B], F32, tag="ck")
                nc.tensor.matmul(z2_ps[:], W2_s[:, :], z1d[:],
                                 start=True, stop=False)
                nc.tensor.matmul(z2_ps[:], b2_s[0:1, :], ones_b[0:1, :],
                                 start=False, stop=True)
                z2d = dice(z2_ps, D // 2)

                s_ps = pck.tile([1, B], F32, tag="ck")
                nc.tensor.matmul(s_ps[:], Wf_s[:, 0:1], z2d[:],
                                 start=True, stop=False)
                nc.tensor.matmul(s_ps[:], bf_s[0:1, 0:1], ones_b[0:1, :],
                                 start=False, stop=True)
                s_sb = wk.tile([1, B], F32, tag="s_sb")
                nc.vector.tensor_copy(s_sb[:], s_ps[:])

                e2 = wk.tile([1, B], F32, tag="e2")
                nc.scalar.activation(e2[:], s_sb[:], AF.Exp)
                nc.vector.tensor_scalar_add(out=e2[:], in0=e2[:], scalar1=1.0)
                sp2 = wk.tile([1, B], F32, tag="sp2")
                nc.scalar.activation(sp2[:], e2[:], AF.Ln)
                spm2 = wk.tile([1, B], F32, tag="spm2")
                nc.vector.tensor_tensor(out=spm2[:], in0=sp2[:], in1=s_sb[:],
                                        op=OP.subtract)
                nc.vector.tensor_scalar_min(out=spm2[:], in0=spm2[:],
                                            scalar1=100.0)
                nc.vector.tensor_scalar_min(out=sp2[:], in0=sp2[:], scalar1=100.0)
                nc.vector.tensor_tensor(out=spm2[:], in0=spm2[:], in1=sp2[:],
                                        op=OP.subtract)
                nc.vector.tensor_tensor(out=spm2[:], in0=y_t_s[:], in1=spm2[:],
                                        op=OP.mult)
                nc.vector.tensor_tensor(out=sp2[:], in0=sp2[:], in1=spm2[:],
                                        op=OP.add)
                rec_sum = wk.tile([1, 1], F32, tag="rec_sum")
                nc.vector.reduce_sum(out=rec_sum[:], in_=sp2[:],
                                     axis=mybir.AxisListType.X)

                nc.vector.tensor_scalar_mul(out=aux_tot[:], in0=aux_tot[:],
                                            scalar1=ALPHA / (B * L))
                nc.vector.tensor_scalar_mul(out=rec_sum[:], in0=rec_sum[:],
                                            scalar1=1.0 / B)
                res = wk.tile([1, 1], F32, tag="res")
                nc.vector.tensor_tensor(out=res[:], in0=aux_tot[:],
                                        in1=rec_sum[:], op=OP.add)
                nc.sync.dma_start(out=out_p[:], in_=res[:])
    nc.compile()
    return nc


_NC_CACHE = None


def _get_nc():
    global _NC_CACHE
    if _NC_CACHE is None:
        import os
        _NC_CACHE = build_bass(os.environ.get("KERNEL_UPTO", "full"))
    return _NC_CACHE


def _prep_inputs(inputs):
    f32, bf = np.float32, np.dtype("bfloat16") if hasattr(np, "bfloat16") else None
    import ml_dtypes
    bf16 = ml_dtypes.bfloat16
    emb = np.ascontiguousarray(inputs["emb"], dtype=f32)
    seqs = np.asarray(inputs["history_seqs"])
    labs = np.asarray(inputs["history_labels"])
    tgt = np.asarray(inputs["target_item"])
    tl = np.asarray(inputs["target_label"]).astype(f32)

    w_ih = np.asarray(inputs["w_ih"], dtype=f32)   # rows: [r | z | n]
    w_hh = np.asarray(inputs["w_hh"], dtype=f32)
    b_ih = np.asarray(inputs["b_ih"], dtype=f32)
    b_hh = np.asarray(inputs["b_hh"], dtype=f32)
    # gate order in banks: [zbar | r]; zbar = negated z
    wgx = np.concatenate([-w_ih[D:2 * D].T, w_ih[0:D].T], axis=1)
    wgh = np.concatenate([-w_hh[D:2 * D].T, w_hh[0:D].T], axis=1)
    bg = np.concatenate([-(b_ih[D:2 * D] + b_hh[D:2 * D]),
                         b_ih[0:D] + b_hh[0:D]]).reshape(1, 2 * D)
    wnx = np.ascontiguousarray(w_ih[2 * D:3 * D].T)
    wnh = np.ascontiguousarray(w_hh[2 * D:3 * D].T)
    bihn = b_ih[2 * D:].reshape(D, 1)
    bhhn = b_hh[2 * D:].reshape(D, 1)

    Wu, Wr, Wh = (np.asarray(inputs[k], dtype=f32) for k in ("Wu", "Wr", "Wh"))
    Uu, Ur, Uh = (np.asarray(inputs[k], dtype=f32) for k in ("Uu", "Ur", "Uh"))
    bu = np.asarray(inputs["bu"], dtype=f32).reshape(-1)
    br = np.asarray(inputs["br"], dtype=f32).reshape(-1)
    bh = np.asarray(inputs["bh"], dtype=f32).reshape(D, 1)
    aux_wm = np.concatenate([Wu, Wr], axis=1)       # [D, 2D] lhsT as-is
    auh_wm = np.concatenate([Uu, Ur], axis=1)
    ba = np.concatenate([bu, br]).reshape(1, 2 * D)

    W1 = np.ascontiguousarray(inputs["W1"], dtype=f32)
    b1 = np.asarray(inputs["b1"], dtype=f32).reshape(1, D)
    W2 = np.ascontiguousarray(inputs["W2"], dtype=f32)
    b2 = np.asarray(inputs["b2"], dtype=f32).reshape(1, D // 2)
    Wf = np.ascontiguousarray(inputs["Wf"], dtype=f32)
    bfv = np.asarray(inputs["bf"], dtype=f32).reshape(1, 1)
    h0 = np.asarray(inputs["h0"], dtype=f32)
    y_t_full = tl.reshape(1, B)

    cvt = lambda a: np.ascontiguousarray(a).astype(bf16)
    shared = dict(
        emb=emb, wgx=cvt(wgx), wgh=cvt(wgh), wnx=cvt(wnx), wnh=cvt(wnh),
        bg_row=cvt(bg), bihn_r=cvt(bihn.reshape(1, D)),
        bhhn_c=bhhn.reshape(D, 1),
        aux_w=cvt(aux_wm), auh_w=cvt(auh_wm), ahx_w=cvt(Wh), ahh_w=cvt(Uh),
        ba_row=cvt(ba), bh_r=cvt(bh.reshape(1, D)),
        W1a=np.ascontiguousarray(W1[0:D]), W1b=np.ascontiguousarray(W1[D:2 * D]),
        b1=b1, W2=W2, b2=b2, Wf=Wf, bf=bfv, y_t=y_t_full)
    in_maps = []
    for c in range(NCORES):
        sl = slice(c * BL, (c + 1) * BL)
        idx_f = np.ascontiguousarray(seqs[sl].T).reshape(-1)
        idx_hc = np.ascontiguousarray(
            idx_f.reshape(NTIL, 128).T).astype(np.int32)
        y_f = np.ascontiguousarray(labs[sl, :, 0].T).reshape(-1).astype(f32)
        y_hc = np.ascontiguousarray(y_f.reshape(NTIL, 128).T)
        h0T = cvt(h0[sl].T)
        idx_tc = tgt[sl].reshape(BL, 1).astype(np.int32)
        m = dict(shared)
        m.update(idx_h=idx_hc, y_h=y_hc, idx_t=idx_tc, h0T=h0T)
        in_maps.append(m)
    return in_maps


def kernel(**inputs) -> np.ndarray:
    nc = _get_nc()
    in_maps = _prep_inputs(inputs)
    res = run_bass_kernel_spmd(nc, in_maps, core_ids=list(range(NCORES)))
    out = np.asarray(res.results[0]["out"], dtype=np.float32)
    return out.reshape(())
